# revision 16
# baseline (speedup 1.0000x reference)
"""Trainium2 Bass kernel for HeatmapMaxDetBlock (argmax + local refinement).

Computes, for x[B, C, H, W]:
    scores = max over (H*W); idx = argmax; px = idx % W, py = idx // W (masked
    by score > 0); quarter-pixel refinement by sign of neighbor differences.
Returns [B, C, 3] = (px, py, scores).

Strategy (pure data parallel over 8 NeuronCores, batch-sharded; 136 heatmap
rows of H*W=49152 f32 per core):
  Rows live DIRECTLY on SBUF partitions. Main group = rows 0..127 on 128
  partitions; the free dim streams the 49152 row columns in chunks (24 KiB
  contiguous per partition per DMA -> near-peak HBM bandwidth). One DVE
  reduce per chunk produces per-(row, segment) maxima with segment width
  W=192, so the winning segment IS py and the in-segment argmax IS px --
  no transpose, no relayout, no integer division.
  The 8 leftover rows (128..135) stream first as a [128, 3072] tile
  (16 partitions per row), get a tiny linearizing SBUF->SBUF DMA into
  [8, 256] segment-max form, and their whole detection chain hides under
  the main stream. Only the main group's short chain (segment argmax ->
  window gather -> max_index -> neighbor gather -> refinement) is exposed
  after the last chunk lands.
"""

import sys
from contextlib import ExitStack
from dataclasses import dataclass

import numpy as np

for _p in ("/opt/trn_rl_repo",):
    if _p not in sys.path:
        sys.path.insert(0, _p)

import concourse.bass as bass  # noqa: E402
import concourse.tile as tile  # noqa: E402
from concourse import bacc, mybir  # noqa: E402

F32 = mybir.dt.float32
I32 = mybir.dt.int32
U32 = mybir.dt.uint32
AX = mybir.AxisListType
OP = mybir.AluOpType


@dataclass(frozen=True)
class Cfg:
    B: int = 64
    C: int = 17
    H: int = 256
    W: int = 192
    ncores: int = 8
    P: int = 128
    FRONT: int = 256
    REAR: int = 512

    @property
    def BP(self):  # batches per core
        return self.B // self.ncores

    @property
    def R(self):  # heatmap rows per core
        return self.BP * self.C

    @property
    def HWm(self):
        return self.H * self.W

    @property
    def NSEG(self):  # segments per row (segment = one heatmap line)
        return self.H

    @property
    def NBW(self):  # neighborhood gather width: [-W .. +W]
        return 2 * self.W + 1

    @property
    def SHN(self):
        return self.R * self.HWm

    @property
    def NPAD(self):
        return self.FRONT + self.SHN + self.REAR

    @property
    def RUMP(self):  # leftover rows beyond the 128-partition main group
        return self.R - self.P


CFG = Cfg()

# Column chunking of the main group's stream: small chunks first so the
# DVE reduce pipeline starts early, big chunks in the middle for DMA
# efficiency, small final chunks so the last reduce is off the critical
# path. The last NDED chunks get dedicated SBUF tiles so their DMAs are
# never issue-gated on reduce progress at the end of the stream.
CHUNKS = [768, 1536, 3072, 3072] + [5760] * 6 + [3072, 1536, 768, 384, 384]
NDED = 5
assert sum(CHUNKS) == CFG.HWm and all(c % CFG.W == 0 for c in CHUNKS)


def build_program(cfg: Cfg):
    c = cfg
    W = c.W
    assert c.RUMP * 16 == c.P, "rump layout assumes 16 partitions per row"
    assert c.FRONT >= W and c.REAR >= 2 * W

    nc = bacc.Bacc(
        "TRN2", target_bir_lowering=False, debug=False, num_devices=c.ncores
    )
    xh = nc.dram_tensor("x", [c.NPAD], F32, kind="ExternalInput").ap()
    oh = nc.dram_tensor("out", [c.R, 3], F32, kind="ExternalOutput").ap()

    with ExitStack() as ctx:
        tc = ctx.enter_context(tile.TileContext(nc))
        xpool = ctx.enter_context(tc.tile_pool(name="xp", bufs=6))
        sp = ctx.enter_context(tc.tile_pool(name="sp", bufs=1))

        # ---- stream DMAs -------------------------------------------------
        # main-group chunk DMAs, alternating the two HWDGE queues
        xts = []
        col = 0
        for i, ncols in enumerate(CHUNKS):
            if i < len(CHUNKS) - NDED:
                xt = xpool.tile([c.P, max(CHUNKS)], F32, tag="xt")
            else:
                xt = sp.tile([c.P, ncols], F32, tag=f"xd{i}")
            src = bass.AP(
                xh.tensor, c.FRONT + col, [[c.HWm, c.P], [1, ncols]]
            )
            eng = nc.sync if i % 2 == 0 else nc.scalar
            eng.dma_start(out=xt[:, 0:ncols], in_=src)
            xts.append((xt, col, ncols))
            col += ncols

        # rump rows ride the (otherwise idle) SWDGE queue; their phase 2
        # hides under the main stream
        xtr = sp.tile([c.P, 3072], F32, tag="xtr")
        rsrc = bass.AP(
            xh.tensor,
            c.FRONT + c.P * c.HWm,
            [[c.HWm, c.RUMP], [3072, 16], [1, 3072]],
        )
        nc.gpsimd.dma_start(out=xtr[:], in_=rsrc)

        # ---- on-chip constants (cheap; off the critical path) ------------
        # rowbase2[p] = FRONT + row_p*HWm + (NSEG-1)*W  (f32-exact, < 2^24)
        base0 = c.FRONT + (c.NSEG - 1) * W
        rb_i = sp.tile([c.P, 1], I32, tag="rb_i")
        nc.gpsimd.iota(rb_i[:], pattern=[[0, 1]], base=base0,
                       channel_multiplier=c.HWm)
        rb_g = sp.tile([c.P, 1], F32, tag="rb_g")
        nc.vector.tensor_copy(out=rb_g[:], in_=rb_i[:])
        rbr_i = sp.tile([c.RUMP, 1], I32, tag="rbr_i")
        nc.gpsimd.iota(rbr_i[:], pattern=[[0, 1]],
                       base=base0 + c.P * c.HWm, channel_multiplier=c.HWm)
        rb_r = sp.tile([c.RUMP, 1], F32, tag="rb_r")
        nc.vector.tensor_copy(out=rb_r[:], in_=rbr_i[:])
        # reversed iota over segments: irt[p, s] = NSEG-1 - s
        irt_i = sp.tile([c.P, c.NSEG], I32, tag="irt_i")
        nc.gpsimd.iota(irt_i[:], pattern=[[-1, c.NSEG]], base=c.NSEG - 1,
                       channel_multiplier=0)
        irt = sp.tile([c.P, c.NSEG], F32, tag="irt")
        nc.vector.tensor_copy(out=irt[:], in_=irt_i[:])
        # interior upper bounds (px < W-1, py < H-1)
        hi2 = sp.tile([c.P, 2], F32, tag="hi2")
        nc.vector.memset(hi2[:, 0:1], float(W - 1))
        nc.vector.memset(hi2[:, 1:2], float(c.H - 1))

        # ---- phase 1 reduces --------------------------------------------
        Mr = sp.tile([c.P, 16], F32, tag="Mr")
        M = sp.tile([c.P, c.NSEG], F32, tag="M")

        def reduce_chunk(i):
            xt, col, ncols = xts[i]
            s0 = col // W
            ns = ncols // W
            nc.vector.reduce_max(
                out=M[:, s0 : s0 + ns],
                in_=xt[:, 0:ncols].rearrange("p (s u) -> p s u", u=W),
                axis=AX.X,
            )

        # rump relayout: [128,16] partition-major -> [8, 256] rows-on-partitions
        R8 = sp.tile([c.RUMP, c.NSEG], F32, tag="R8")

        # ---- phase 2 chain (per group), in three parts -------------------
        # part 1: segment argmax -> window gather issued
        # part 2: in-window max_index -> neighborhood gather issued
        # part 3: px/py assembly + quarter-pixel refinement
        def chain_p1(Mg, rb, gp, tagp):
            st = {}
            scores = sp.tile([gp, 1], F32, tag=f"sc{tagp}")
            nc.vector.reduce_max(out=scores[:], in_=Mg, axis=AX.X)
            mk = sp.tile([gp, c.NSEG], F32, tag=f"mk{tagp}")
            nc.vector.tensor_tensor(
                out=mk[:], in0=Mg,
                in1=scores[:].to_broadcast([gp, c.NSEG]), op=OP.is_equal,
            )
            nc.vector.tensor_tensor(
                out=mk[:], in0=mk[:], in1=irt[0:gp], op=OP.mult
            )
            srev = sp.tile([gp, 1], F32, tag=f"sr{tagp}")
            nc.vector.reduce_max(out=srev[:], in_=mk[:], axis=AX.X)
            # window start (absolute in padded x): w0 = rb - W*srev
            w0 = sp.tile([gp, 1], F32, tag=f"w0{tagp}")
            nc.vector.tensor_scalar(
                out=w0[:], in0=srev[:], scalar1=-float(W), scalar2=None,
                op0=OP.mult,
            )
            nc.vector.tensor_tensor(out=w0[:], in0=w0[:], in1=rb[:], op=OP.add)
            w0u = sp.tile([gp, 1], U32, tag=f"w0u{tagp}")
            nc.vector.tensor_copy(out=w0u[:], in_=w0[:])
            win = sp.tile([gp, W], F32, tag=f"win{tagp}")
            nc.gpsimd.indirect_dma_start(
                out=win[:],
                out_offset=None,
                in_=xh[:, None],
                in_offset=bass.IndirectOffsetOnAxis(ap=w0u[:, 0:1], axis=0),
            )
            st.update(scores=scores, srev=srev, w0=w0, win=win)
            return st

        def chain_p2(st, gp, tagp):
            scores, w0, win = st["scores"], st["w0"], st["win"]
            m8 = sp.tile([gp, 8], F32, tag=f"m8{tagp}")
            nc.vector.tensor_copy(out=m8[:], in_=scores[:].to_broadcast([gp, 8]))
            mi = sp.tile([gp, 8], U32, tag=f"mi{tagp}")
            nc.vector.max_index(mi[:], m8[:], win[:])
            ii = sp.tile([gp, 1], F32, tag=f"ii{tagp}")
            nc.vector.tensor_copy(out=ii[:], in_=mi[:, 0:1])

            # neighborhood gather: start = peak - W = w0 + ii - W
            w2 = sp.tile([gp, 1], F32, tag=f"w2{tagp}")
            nc.vector.tensor_tensor(out=w2[:], in0=w0[:], in1=ii[:], op=OP.add)
            nc.vector.tensor_scalar(
                out=w2[:], in0=w2[:], scalar1=-float(W),
                scalar2=float(c.NPAD - c.NBW), op0=OP.add, op1=OP.min,
            )
            w2u = sp.tile([gp, 1], U32, tag=f"w2u{tagp}")
            nc.vector.tensor_copy(out=w2u[:], in_=w2[:])
            nb = sp.tile([gp, c.NBW], F32, tag=f"nb{tagp}")
            nc.gpsimd.indirect_dma_start(
                out=nb[:],
                out_offset=None,
                in_=xh[:, None],
                in_offset=bass.IndirectOffsetOnAxis(ap=w2u[:, 0:1], axis=0),
            )
            st.update(ii=ii, nb=nb)

        def chain_p3(st, gp, tagp):
            scores, srev, ii, nb = st["scores"], st["srev"], st["ii"], st["nb"]
            # px = ii, py = NSEG-1 - srev, masked by score > 0
            O = sp.tile([gp, 3], F32, tag=f"O{tagp}")
            sv = sp.tile([gp, 1], F32, tag=f"sv{tagp}")
            nc.vector.tensor_scalar(
                out=sv[:], in0=srev[:], scalar1=-1.0,
                scalar2=float(c.NSEG - 1), op0=OP.mult, op1=OP.add,
            )
            mkp = sp.tile([gp, 1], F32, tag=f"mkp{tagp}")
            nc.vector.tensor_scalar(
                out=mkp[:], in0=scores[:], scalar1=0.0, scalar2=None,
                op0=OP.is_gt,
            )
            nc.vector.tensor_tensor(
                out=O[:, 0:1], in0=ii[:], in1=mkp[:], op=OP.mult
            )
            nc.vector.tensor_tensor(
                out=O[:, 1:2], in0=sv[:], in1=mkp[:], op=OP.mult
            )
            # interior = (0 < px < W-1) & (0 < py < H-1)
            ilo = sp.tile([gp, 2], F32, tag=f"ilo{tagp}")
            nc.vector.tensor_scalar(
                out=ilo[:], in0=O[:, 0:2], scalar1=0.0, scalar2=None,
                op0=OP.is_gt,
            )
            ihi = sp.tile([gp, 2], F32, tag=f"ihi{tagp}")
            nc.vector.tensor_tensor(
                out=ihi[:], in0=O[:, 0:2], in1=hi2[0:gp], op=OP.is_lt
            )
            nc.vector.tensor_tensor(out=ilo[:], in0=ilo[:], in1=ihi[:], op=OP.mult)
            intr = sp.tile([gp, 1], F32, tag=f"in{tagp}")
            nc.vector.tensor_reduce(out=intr[:], in_=ilo[:], axis=AX.X, op=OP.min)

            # dx = sign(nb[W+1] - nb[W-1]); dy = sign(nb[2W] - nb[0])
            D = sp.tile([gp, 2], F32, tag=f"D{tagp}")
            DL = sp.tile([gp, 2], F32, tag=f"DL{tagp}")
            for a, (ir, il) in enumerate(((W + 1, W - 1), (2 * W, 0))):
                nc.vector.tensor_tensor(
                    out=D[:, a : a + 1], in0=nb[:, ir : ir + 1],
                    in1=nb[:, il : il + 1], op=OP.is_gt,
                )
                nc.vector.tensor_tensor(
                    out=DL[:, a : a + 1], in0=nb[:, ir : ir + 1],
                    in1=nb[:, il : il + 1], op=OP.is_lt,
                )
            nc.vector.tensor_tensor(out=D[:], in0=D[:], in1=DL[:], op=OP.subtract)
            nc.vector.tensor_scalar(
                out=D[:], in0=D[:], scalar1=0.25, scalar2=None, op0=OP.mult
            )
            nc.vector.tensor_tensor(
                out=D[:], in0=D[:], in1=intr[:].to_broadcast([gp, 2]), op=OP.mult
            )
            nc.vector.tensor_tensor(out=O[:, 0:2], in0=O[:, 0:2], in1=D[:], op=OP.add)
            nc.vector.tensor_copy(out=O[:, 2:3], in_=scores[:])
            return O

        # ---- emission schedule ------------------------------------------
        # The stream phase is pure chunk reduces on DVE -- nothing else is
        # allowed to delay them, or ring-buffer reuse gates the chunk DMA
        # issues and the whole stream lock-steps. All phase-2 work happens
        # after the last chunk, with the rump chain interleaved into the
        # main chain's DMA-wait gaps (each chain's indirect-gather latency
        # is covered by the other chain's compute).
        for i in range(len(CHUNKS)):
            reduce_chunk(i)
        st_g = chain_p1(M[:], rb_g, c.P, "g")  # argseg + win_g gather
        nc.vector.reduce_max(
            out=Mr[:], in_=xtr[:].rearrange("p (s u) -> p s u", u=W), axis=AX.X
        )
        nc.gpsimd.dma_start(out=R8[:], in_=Mr[:])  # linearizing relayout
        st_r = chain_p1(R8[:], rb_r, c.RUMP, "r")
        chain_p2(st_g, c.P, "g")  # max_index + nb_g gather
        chain_p2(st_r, c.RUMP, "r")
        Og = chain_p3(st_g, c.P, "g")
        nc.sync.dma_start(out=oh[0 : c.P], in_=Og[:])
        Or = chain_p3(st_r, c.RUMP, "r")
        nc.gpsimd.dma_start(out=oh[c.P : c.R], in_=Or[:])

    nc.compile()
    return nc


def shard_inputs(cfg: Cfg, x: np.ndarray):
    c = cfg
    in_maps = []
    for k in range(c.ncores):
        shard = np.ascontiguousarray(
            x[k * c.BP : (k + 1) * c.BP], dtype=np.float32
        ).reshape(-1)
        xp = np.zeros(c.NPAD, np.float32)
        xp[c.FRONT : c.FRONT + c.SHN] = shard
        in_maps.append({"x": xp})
    return in_maps


def assemble_out(cfg: Cfg, per_core_outs):
    c = cfg
    outs = [o.reshape(c.BP, c.C, 3).astype(np.float32) for o in per_core_outs]
    return np.concatenate(outs, axis=0)


_PROGRAM = None


def _program():
    global _PROGRAM
    if _PROGRAM is None:
        _PROGRAM = build_program(CFG)
    return _PROGRAM


def kernel(x: np.ndarray) -> np.ndarray:
    from concourse.bass_utils import run_bass_kernel_spmd

    c = CFG
    assert x.shape == (c.B, c.C, c.H, c.W), x.shape
    nc = _program()
    in_maps = shard_inputs(c, np.asarray(x))
    res = run_bass_kernel_spmd(nc, in_maps, core_ids=list(range(c.ncores)))
    return assemble_out(c, [res.results[k]["out"] for k in range(c.ncores)])
